# revision 5
# baseline (speedup 1.0000x reference)
"""MetaConvSmoother Trainium2 kernel (Bass/Tile), data-parallel over 8 NeuronCores.

Per core (8 samples):
  - hypernet MLPs (9 -> 100 -> 147, exact gelu) on PE + ACT
  - per-sample conv kernels staged as zero-padded tables in DRAM
    (one 255-float table per (sample, tap-column); U[127-ky] = w[ky, kx])
  - flipped Toeplitz bands Bf[i, m] = U[i+m] loaded with all-positive strided
    DMAs, then partition-reversed on the TensorEngine with a constant
    anti-diagonal matrix: B[p, m] = U[127 + m - p]
  - each conv stage = banded matmuls over image rows (lhsT = B slices),
    column taps via free-dim offset reads of the rhs tile, PSUM accumulation:
      Ax   : 3x3, asymmetric pad (top/left 0, bottom/right 1.0)
      tmp_m: 7x7 corr of r = f - Ax        (3 maps)
      G2   : sum_m 7x7 corr of tmp_m
      out  = x + G2
  - r and tmp round-trip through DRAM to decouple row-tile alignments
  - DMA engine split to avoid FIFO head-of-line blocking:
      SP   : independent loads (x, f, bands)
      POOL : dependent loads (r-in, tmp-in) + table scatter writes (SWDGE)
      ACT  : stores (r-out, tmp-out, out) + band-reversal PSUM->SBUF copies
      DVE  : sub/add/copies/memsets
"""
import numpy as np

import concourse.bass as bass
import concourse.mybir as mybir
from concourse import bacc, bass_utils
from concourse.tile import TileContext

F32 = mybir.dt.float32
S = 8          # samples per core
N = 512
ML = 3
KK = 7
NCORES = 8

# table layout (elements) in the flat DRAM "tables" tensor
TBL = 255
BASE_A = 0                      # (s, kx)        -> 8*3 tables
BASE_S1 = 24 * TBL              # (s, m, kx)     -> 8*21
BASE_S2 = BASE_S1 + 168 * TBL
TBL_TOTAL = BASE_S2 + 168 * TBL  # 91800 elements

NSLOT = 45                      # band slots per sample: 3 A + 21 S1 + 21 S2
BANDW = 128                     # cols per band slot
BF = NSLOT * BANDW              # 5760

# row tilings (out_row_start, M, input_row_start)
AX_TILES = [(0, 126, -1), (126, 126, 125), (252, 126, 251), (378, 126, 377),
            (504, 8, 503)]
S7_TILES = [(0, 122, -3), (122, 122, 119), (244, 122, 241), (366, 122, 363),
            (488, 24, 485)]


def _sub_ap(base_ap, pattern, offset):
    """Custom access-pattern view: list of [step, count] pairs + elem offset."""
    a = base_ap.copy()
    v = a.ap
    v.clear()
    for p in pattern:
        v.append(list(p))
    a.offset = base_ap.offset + offset
    return a


def _slot_a(kx):
    return kx


def _slot_s1(m, kx):
    return 3 + m * KK + kx


def _slot_s2(m, kx):
    return 24 + m * KK + kx


def build_kernel(nc):
    x = nc.dram_tensor("x", [S, N, N], F32, kind="ExternalInput").ap()
    f = nc.dram_tensor("f", [S, N, N], F32, kind="ExternalInput").ap()
    ka = nc.dram_tensor("kernelA", [S, 9], F32, kind="ExternalInput").ap()
    fc_w1 = [nc.dram_tensor(f"fc{i}_w1", [100, 9], F32, kind="ExternalInput").ap()
             for i in (1, 2)]
    fc_b1 = [nc.dram_tensor(f"fc{i}_b1", [100], F32, kind="ExternalInput").ap()
             for i in (1, 2)]
    fc_w2 = [nc.dram_tensor(f"fc{i}_w2", [147, 100], F32, kind="ExternalInput").ap()
             for i in (1, 2)]
    fc_b2 = [nc.dram_tensor(f"fc{i}_b2", [147], F32, kind="ExternalInput").ap()
             for i in (1, 2)]
    out = nc.dram_tensor("out", [S, N, N], F32, kind="ExternalOutput").ap()

    with TileContext(nc) as tc:
        with (
            tc.tile_pool(name="dram", bufs=1, space="DRAM") as dpool,
            tc.tile_pool(name="const", bufs=1) as cpool,
            tc.tile_pool(name="mlp", bufs=1) as mpool,
            tc.tile_pool(name="bandf", bufs=1) as bfpool,
            tc.tile_pool(name="bands", bufs=2) as bpool,
            tc.tile_pool(name="xa", bufs=6) as xa_pool,
            tc.tile_pool(name="fr", bufs=4) as fr_pool,
            tc.tile_pool(name="rhs7", bufs=4) as rhs_pool,
            tc.tile_pool(name="stout", bufs=4) as st_pool,
            tc.tile_pool(name="psA", bufs=2, space="PSUM") as psA,
            tc.tile_pool(name="ps1", bufs=2, space="PSUM") as ps1,
            tc.tile_pool(name="ps2", bufs=2, space="PSUM") as ps2,
            tc.tile_pool(name="psx", bufs=2, space="PSUM") as psx,
        ):
            tables = dpool.tile([TBL_TOTAL], F32)
            r_dram = dpool.tile([S, N, N], F32)
            tmp_dram = dpool.tile([S, ML, N, N], F32)

            # ---- constants: anti-diagonal reversal matrix Rev[k,p]=d(k+p=127)
            rev = cpool.tile([128, 128], F32)
            nc.gpsimd.memset(rev, 0.0)
            nc.gpsimd.affine_select(
                out=rev, in_=rev, compare_op=mybir.AluOpType.not_equal,
                fill=1.0, base=-127, pattern=[[1, 128]], channel_multiplier=1)

            # ---- zero-fill tables
            zt = cpool.tile([120, 765], F32)
            nc.vector.memset(zt, 0.0)
            nc.sync.dma_start(_sub_ap(tables, [[765, 120], [1, 765]], 0), zt)

            # ---------------- MLP + weight staging ----------------
            ident = cpool.tile([128, 128], F32)
            nc.gpsimd.memset(ident, 0.0)
            nc.gpsimd.affine_select(
                out=ident, in_=ident, compare_op=mybir.AluOpType.not_equal,
                fill=1.0, base=0, pattern=[[-1, 128]], channel_multiplier=1)

            vT = mpool.tile([9, S], F32)
            nc.sync.dma_start(vT, ka.rearrange("s k -> k s"))

            w_sb = {}  # (layer i, map m) -> [49, S] conv weights
            for i in range(2):
                w1n = mpool.tile([100, 9], F32, name=f"w1n{i}")
                nc.sync.dma_start(w1n, fc_w1[i])
                W1T = mpool.tile([9, 100], F32, name=f"W1T{i}")
                t1 = psx.tile([9, 100], F32, name=f"t1_{i}", tag="aux")
                nc.tensor.transpose(t1, w1n, ident[:100, :100])
                nc.vector.tensor_copy(W1T, t1)

                b1 = mpool.tile([100, 1], F32, name=f"b1_{i}")
                nc.sync.dma_start(b1, fc_b1[i].unsqueeze(1))

                w2n_a = mpool.tile([128, 100], F32, name=f"w2na{i}")
                nc.sync.dma_start(w2n_a, fc_w2[i][0:128, :])
                w2n_b = mpool.tile([19, 100], F32, name=f"w2nb{i}")
                nc.sync.dma_start(w2n_b, fc_w2[i][128:147, :])
                W2T = mpool.tile([100, 147], F32, name=f"W2T{i}")
                tr_a = psx.tile([100, 128], F32, name=f"tra{i}", tag="aux")
                nc.tensor.transpose(tr_a, w2n_a, ident)
                nc.vector.tensor_copy(W2T[:, 0:128], tr_a)
                tr_b = psx.tile([100, 19], F32, name=f"trb{i}", tag="aux")
                nc.tensor.transpose(tr_b, w2n_b, ident[:19, :19])
                nc.vector.tensor_copy(W2T[:, 128:147], tr_b)

                h_pre = psx.tile([100, S], F32, name=f"hpre{i}", tag="aux")
                nc.tensor.matmul(h_pre, W1T, vT, start=True, stop=True)
                h = mpool.tile([100, S], F32, name=f"h{i}")
                nc.scalar.activation(
                    h, h_pre, mybir.ActivationFunctionType.Gelu, bias=b1)

                for m in range(ML):
                    b2m = mpool.tile([49, 1], F32, name=f"b2_{i}_{m}")
                    nc.sync.dma_start(
                        b2m, fc_b2[i][49 * m:49 * m + 49].unsqueeze(1))
                    wp = psx.tile([49, S], F32, name=f"wp{i}{m}", tag="aux")
                    nc.tensor.matmul(wp, W2T[:, 49 * m:49 * m + 49], h,
                                     start=True, stop=True)
                    wsb = mpool.tile([49, S], F32, name=f"w_{i}_{m}")
                    nc.scalar.activation(
                        wsb, wp, mybir.ActivationFunctionType.Identity,
                        bias=b2m)
                    w_sb[(i, m)] = wsb

            # scatter conv weights into zero-padded tables (SWDGE, flexible)
            # A tables: U[(s*3+kx)*255 + 127 - ky] = kernelA[s, ky, kx]
            for ky in range(3):
                nc.gpsimd.dma_start(
                    _sub_ap(tables, [[TBL, 3], [3 * TBL, S]],
                            BASE_A + 127 - ky),
                    vT[3 * ky:3 * ky + 3, :])
            # stage1/2: U[((s*3+m)*7+kx)*255 + 127 - ky] = w[i][s, m, ky, kx]
            for i, base in ((0, BASE_S1), (1, BASE_S2)):
                for m in range(ML):
                    for ky in range(KK):
                        nc.gpsimd.dma_start(
                            _sub_ap(tables, [[TBL, KK], [21 * TBL, S]],
                                    base + m * KK * TBL + 127 - ky),
                            w_sb[(i, m)][KK * ky:KK * ky + KK, :])

            # ---------------- main per-sample loop ----------------
            for s in range(S):
                # ---- flipped bands Bf[i, slot, m] = U_slot[i + m]
                bf = bfpool.tile([128, BF], F32, name=f"bf{s}", tag="bf")
                for (nslots, slot0, base) in (
                        (3, 0, BASE_A + s * 3 * TBL),
                        (21, 3, BASE_S1 + s * 21 * TBL),
                        (21, 24, BASE_S2 + s * 21 * TBL)):
                    nc.sync.dma_start(
                        _sub_ap(bf, [[BF, 128], [BANDW, nslots], [1, BANDW]],
                                slot0 * BANDW),
                        _sub_ap(tables, [[1, 128], [TBL, nslots], [1, BANDW]],
                                base))
                # ---- reverse partitions on PE: B[p] = Bf[127-p]
                bb = bpool.tile([128, BF], F32, name=f"bb{s}", tag="bands")
                for c in range(0, BF, 512):
                    w = min(512, BF - c)
                    pr = psx.tile([128, 512], F32, name=f"pr{s}_{c}", tag="aux")
                    nc.tensor.matmul(pr[:, :w], rev, bf[:, c:c + w],
                                     start=True, stop=True)
                    nc.scalar.copy(bb[:, c:c + w], pr[:, :w])

                def band(slot, M):
                    return bb[:, slot * BANDW:slot * BANDW + M]

                # ---- Ax and r = f - Ax (126-row tiles) ----
                for (o0, M, row_start) in AX_TILES:
                    xt = xa_pool.tile([128, N + 2], F32, name=f"xt{s}_{o0}",
                                      tag="xa")
                    if row_start + 128 > N:          # bottom tile: ones pad
                        nc.vector.memset(xt, 1.0)
                        nd = N - row_start
                        nc.sync.dma_start(xt[0:nd, 1:N + 1],
                                          x[s, row_start:N, :])
                        nc.vector.memset(xt[0:nd, 0:1], 0.0)
                    else:
                        lo = max(0, row_start)
                        p0 = lo - row_start
                        if p0 > 0:
                            nc.vector.memset(xt[0:p0, :], 0.0)
                        nc.sync.dma_start(xt[p0:128, 1:N + 1],
                                          x[s, lo:row_start + 128, :])
                        nc.vector.memset(xt[:, 0:1], 0.0)
                        nc.vector.memset(xt[:, N + 1:N + 2], 1.0)
                    ps = psA.tile([M, N], F32, name=f"psA{s}_{o0}", tag="ax")
                    for kx in range(3):
                        nc.tensor.matmul(ps, band(_slot_a(kx), M),
                                         xt[:, kx:kx + N],
                                         start=(kx == 0), stop=(kx == 2))
                    ft = fr_pool.tile([126, N], F32, name=f"ft{s}_{o0}",
                                      tag="f")
                    nc.sync.dma_start(ft[:M, :], f[s, o0:o0 + M, :])
                    rt = fr_pool.tile([126, N], F32, name=f"rt{s}_{o0}",
                                      tag="r")
                    nc.vector.tensor_sub(rt[:M, :], ft[:M, :], ps)
                    nc.scalar.dma_start(r_dram[s, o0:o0 + M, :], rt[:M, :])

                # ---- stage 1: tmp_m = corr7(r, w1_m) ----
                for (o0, M, row_start) in S7_TILES:
                    rt7 = rhs_pool.tile([128, N + 6], F32,
                                        name=f"rt7_{s}_{o0}", tag="rt7")
                    nc.vector.memset(rt7, 0.0)
                    lo = max(0, row_start)
                    hi = min(N, row_start + 128)
                    nc.gpsimd.dma_start(
                        rt7[lo - row_start:hi - row_start, 3:N + 3],
                        r_dram[s, lo:hi, :])
                    tm3 = st_pool.tile([122, 3 * N], F32,
                                       name=f"tm3_{s}_{o0}", tag="tmp")
                    for m in range(ML):
                        ps = ps1.tile([M, N], F32, name=f"ps1_{s}_{o0}_{m}",
                                      tag="s1")
                        for kx in range(KK):
                            nc.tensor.matmul(ps, band(_slot_s1(m, kx), M),
                                             rt7[:, kx:kx + N],
                                             start=(kx == 0), stop=(kx == 6))
                        nc.vector.tensor_copy(tm3[:M, m * N:(m + 1) * N], ps)
                    # one store for all 3 maps: tmp_dram[s, :, o0:o0+M, :]
                    nc.scalar.dma_start(
                        _sub_ap(tmp_dram, [[N, M], [N * N, ML], [1, N]],
                                ((s * ML) * N + o0) * N),
                        _sub_ap(tm3, [[3 * N, M], [N, ML], [1, N]], 0))

                # ---- stage 2: G2 = sum_m corr7(tmp_m, w2_m); out = x + G2
                for (o0, M, row_start) in S7_TILES:
                    lo = max(0, row_start)
                    hi = min(N, row_start + 128)
                    tt = rhs_pool.tile([128, 3 * (N + 6)], F32,
                                       name=f"tt{s}_{o0}", tag="tt")
                    nc.vector.memset(tt, 0.0)
                    # one load for all 3 maps, each into its 518-block at col 3
                    nc.gpsimd.dma_start(
                        _sub_ap(tt, [[3 * (N + 6), hi - lo],
                                     [N + 6, ML], [1, N]],
                                (lo - row_start) * 3 * (N + 6) + 3),
                        _sub_ap(tmp_dram, [[N, hi - lo], [N * N, ML], [1, N]],
                                ((s * ML) * N + lo) * N))
                    pg = ps2.tile([M, N], F32, name=f"ps2_{s}_{o0}", tag="s2")
                    idx = 0
                    for m in range(ML):
                        for kx in range(KK):
                            nc.tensor.matmul(
                                pg, band(_slot_s2(m, kx), M),
                                tt[:, m * (N + 6) + kx:m * (N + 6) + kx + N],
                                start=(idx == 0), stop=(idx == 20))
                            idx += 1
                    x2 = fr_pool.tile([126, N], F32, name=f"x2_{s}_{o0}",
                                      tag="x2")
                    nc.sync.dma_start(x2[:M, :], x[s, o0:o0 + M, :])
                    ob = st_pool.tile([122, N], F32, name=f"ob{s}_{o0}",
                                      tag="ob")
                    nc.vector.tensor_add(ob[:M, :], x2[:M, :], pg)
                    nc.scalar.dma_start(out[s, o0:o0 + M, :], ob[:M, :])
    return nc


_CACHED = None


def _get_nc():
    global _CACHED
    if _CACHED is None:
        nc = bacc.Bacc("TRN2", debug=False, enable_asserts=False,
                       num_devices=NCORES)
        build_kernel(nc)
        nc.compile()
        _CACHED = nc
    return _CACHED


def make_in_maps(x, f, kernelA, fc1_w1, fc1_b1, fc1_w2, fc1_b2,
                 fc2_w1, fc2_b1, fc2_w2, fc2_b2):
    shared = {
        "fc1_w1": np.ascontiguousarray(fc1_w1, np.float32),
        "fc1_b1": np.ascontiguousarray(fc1_b1, np.float32),
        "fc1_w2": np.ascontiguousarray(fc1_w2, np.float32),
        "fc1_b2": np.ascontiguousarray(fc1_b2, np.float32),
        "fc2_w1": np.ascontiguousarray(fc2_w1, np.float32),
        "fc2_b1": np.ascontiguousarray(fc2_b1, np.float32),
        "fc2_w2": np.ascontiguousarray(fc2_w2, np.float32),
        "fc2_b2": np.ascontiguousarray(fc2_b2, np.float32),
    }
    in_maps = []
    for c in range(NCORES):
        sl = slice(S * c, S * (c + 1))
        in_maps.append({
            "x": np.ascontiguousarray(x[sl, 0], np.float32),
            "f": np.ascontiguousarray(f[sl, 0], np.float32),
            "kernelA": np.ascontiguousarray(
                kernelA[sl, 0].reshape(S, 9), np.float32),
            **shared,
        })
    return in_maps


def kernel(x, f, kernelA, fc1_w1, fc1_b1, fc1_w2, fc1_b2,
           fc2_w1, fc2_b1, fc2_w2, fc2_b2):
    x = np.asarray(x)
    nc = _get_nc()
    in_maps = make_in_maps(x, f, kernelA, fc1_w1, fc1_b1, fc1_w2, fc1_b2,
                           fc2_w1, fc2_b1, fc2_w2, fc2_b2)
    res = bass_utils.run_bass_kernel_spmd(
        nc, in_maps, core_ids=list(range(NCORES)))
    outs = [res.results[c]["out"] for c in range(NCORES)]
    full = np.concatenate(outs, axis=0).reshape(64, 1, N, N).astype(np.float32)
    return full


# revision 9
# speedup vs baseline: 1.5546x; 1.5546x over previous
"""MetaConvSmoother Trainium2 kernel (Bass/Tile), data-parallel over 8 NeuronCores.

Per core (8 samples):
  - hypernet MLPs (9 -> 100 -> 147, exact gelu) on PE + ACT
  - per-sample conv kernels staged as zero-padded tables in DRAM
    (one 255-float table per (sample, tap-column); U[127-ky] = w[ky, kx])
  - flipped Toeplitz bands Bf[i, m] = U[i+m] loaded with all-positive strided
    DMAs, then partition-reversed on the TensorEngine with a constant
    anti-diagonal matrix: B[p, m] = U[127 + m - p]
  - each conv stage = banded matmuls over image rows (lhsT = B slices),
    column taps via free-dim offset reads of the rhs tile, PSUM accumulation:
      Ax   : 3x3, asymmetric pad (top/left 0, bottom/right 1.0)
      tmp_m: 7x7 corr of r = f - Ax        (3 maps)
      G2   : sum_m 7x7 corr of tmp_m
      out  = x + G2
  - r and tmp round-trip through DRAM to decouple row-tile alignments
  - DMA engine split to avoid FIFO head-of-line blocking:
      SP   : independent loads (x, f, bands)
      POOL : dependent loads (r-in, tmp-in) + table scatter writes (SWDGE)
      ACT  : stores (r-out, tmp-out, out) + band-reversal PSUM->SBUF copies
      DVE  : sub/add/copies/memsets
"""
import numpy as np

import concourse.bass as bass
import concourse.mybir as mybir
from concourse import bacc, bass_utils
from concourse.tile import TileContext

F32 = mybir.dt.float32
F32R = mybir.dt.float32r
USE_F32R = True


def _cast(ap):
    return ap
S = 8          # samples per core
N = 512
ML = 3
KK = 7
NCORES = 8

# table layout (elements) in the flat DRAM "tables" tensor
TBL = 255
BASE_A = 0                      # (s, kx)        -> 8*3 tables
BASE_S1 = 24 * TBL              # (s, m, kx)     -> 8*21
BASE_S2 = BASE_S1 + 168 * TBL
TBL_TOTAL = BASE_S2 + 168 * TBL  # 91800 elements

NSLOT = 45                      # band slots per sample: 3 A + 21 S1 + 21 S2
BANDW = 128                     # cols per band slot
BF = NSLOT * BANDW              # 5760

# row tilings (out_row_start, M, input_row_start)
AX_TILES = [(0, 126, -1), (126, 126, 125), (252, 126, 251), (378, 126, 377),
            (504, 8, 503)]
S7_TILES = [(0, 122, -3), (122, 122, 119), (244, 122, 241), (366, 122, 363),
            (488, 24, 485)]


def _sub_ap(base_ap, pattern, offset):
    """Custom access-pattern view: list of [step, count] pairs + elem offset."""
    a = base_ap.copy()
    v = a.ap
    v.clear()
    for p in pattern:
        v.append(list(p))
    a.offset = base_ap.offset + offset
    return a


def _slot_a(kx):
    return kx


def _slot_s1(m, kx):
    return 3 + m * KK + kx


def _slot_s2(m, kx):
    return 24 + m * KK + kx


def build_kernel(nc):
    x = nc.dram_tensor("x", [S, N, N], F32, kind="ExternalInput").ap()
    f = nc.dram_tensor("f", [S, N, N], F32, kind="ExternalInput").ap()
    ka = nc.dram_tensor("kernelA", [S, 9], F32, kind="ExternalInput").ap()
    fc_w1 = [nc.dram_tensor(f"fc{i}_w1", [100, 9], F32, kind="ExternalInput").ap()
             for i in (1, 2)]
    fc_b1 = [nc.dram_tensor(f"fc{i}_b1", [100], F32, kind="ExternalInput").ap()
             for i in (1, 2)]
    fc_w2 = [nc.dram_tensor(f"fc{i}_w2", [147, 100], F32, kind="ExternalInput").ap()
             for i in (1, 2)]
    fc_b2 = [nc.dram_tensor(f"fc{i}_b2", [147], F32, kind="ExternalInput").ap()
             for i in (1, 2)]
    out = nc.dram_tensor("out", [S, N, N], F32, kind="ExternalOutput").ap()

    with TileContext(nc) as tc:
        with (
            tc.tile_pool(name="dram", bufs=1, space="DRAM") as dpool,
            tc.tile_pool(name="const", bufs=1) as cpool,
            tc.tile_pool(name="mlp", bufs=1) as mpool,
            tc.tile_pool(name="bandf", bufs=1) as bfpool,
            tc.tile_pool(name="bands", bufs=2) as bpool,
            tc.tile_pool(name="xa", bufs=6) as xa_pool,
            tc.tile_pool(name="fr", bufs=4) as fr_pool,
            tc.tile_pool(name="rhs7", bufs=4) as rhs_pool,
            tc.tile_pool(name="stout", bufs=4) as st_pool,
            tc.tile_pool(name="psA", bufs=2, space="PSUM") as psA,
            tc.tile_pool(name="ps1", bufs=2, space="PSUM") as ps1,
            tc.tile_pool(name="ps2", bufs=2, space="PSUM") as ps2,
            tc.tile_pool(name="psx", bufs=2, space="PSUM") as psx,
        ):
            tables = dpool.tile([TBL_TOTAL], F32)
            r_dram = dpool.tile([S, N, N], F32)
            tmp_dram = dpool.tile([S, ML, N, N], F32)

            # ---- constants: anti-diagonal reversal matrix Rev[k,p]=d(k+p=127)
            rev = cpool.tile([128, 128], F32)
            nc.gpsimd.memset(rev, 0.0)
            nc.gpsimd.affine_select(
                out=rev, in_=rev, compare_op=mybir.AluOpType.not_equal,
                fill=1.0, base=-127, pattern=[[1, 128]], channel_multiplier=1)

            # ---- zero-fill tables
            zt = cpool.tile([120, 765], F32)
            nc.vector.memset(zt, 0.0)
            nc.sync.dma_start(_sub_ap(tables, [[765, 120], [1, 765]], 0), zt)

            # ---------------- MLP + weight staging ----------------
            ident = cpool.tile([128, 128], F32)
            nc.gpsimd.memset(ident, 0.0)
            nc.gpsimd.affine_select(
                out=ident, in_=ident, compare_op=mybir.AluOpType.not_equal,
                fill=1.0, base=0, pattern=[[-1, 128]], channel_multiplier=1)

            vT = mpool.tile([9, S], F32)
            nc.sync.dma_start(vT, ka.rearrange("s k -> k s"))

            w_sb = {}  # (layer i, map m) -> [49, S] conv weights
            for i in range(2):
                w1n = mpool.tile([100, 9], F32, name=f"w1n{i}")
                nc.sync.dma_start(w1n, fc_w1[i])
                W1T = mpool.tile([9, 100], F32, name=f"W1T{i}")
                t1 = psx.tile([9, 100], F32, name=f"t1_{i}", tag="aux")
                nc.tensor.transpose(t1, w1n, ident[:100, :100])
                nc.vector.tensor_copy(W1T, t1)

                b1 = mpool.tile([100, 1], F32, name=f"b1_{i}")
                nc.sync.dma_start(b1, fc_b1[i].unsqueeze(1))

                w2n_a = mpool.tile([128, 100], F32, name=f"w2na{i}")
                nc.sync.dma_start(w2n_a, fc_w2[i][0:128, :])
                w2n_b = mpool.tile([19, 100], F32, name=f"w2nb{i}")
                nc.sync.dma_start(w2n_b, fc_w2[i][128:147, :])
                W2T = mpool.tile([100, 147], F32, name=f"W2T{i}")
                tr_a = psx.tile([100, 128], F32, name=f"tra{i}", tag="aux")
                nc.tensor.transpose(tr_a, w2n_a, ident)
                nc.vector.tensor_copy(W2T[:, 0:128], tr_a)
                tr_b = psx.tile([100, 19], F32, name=f"trb{i}", tag="aux")
                nc.tensor.transpose(tr_b, w2n_b, ident[:19, :19])
                nc.vector.tensor_copy(W2T[:, 128:147], tr_b)

                h_pre = psx.tile([100, S], F32, name=f"hpre{i}", tag="aux")
                nc.tensor.matmul(h_pre, W1T, vT, start=True, stop=True)
                h = mpool.tile([100, S], F32, name=f"h{i}")
                nc.scalar.activation(
                    h, h_pre, mybir.ActivationFunctionType.Gelu, bias=b1)

                for m in range(ML):
                    b2m = mpool.tile([49, 1], F32, name=f"b2_{i}_{m}")
                    nc.sync.dma_start(
                        b2m, fc_b2[i][49 * m:49 * m + 49].unsqueeze(1))
                    wp = psx.tile([49, S], F32, name=f"wp{i}{m}", tag="aux")
                    nc.tensor.matmul(wp, W2T[:, 49 * m:49 * m + 49], h,
                                     start=True, stop=True)
                    wsb = mpool.tile([49, S], F32, name=f"w_{i}_{m}")
                    nc.scalar.activation(
                        wsb, wp, mybir.ActivationFunctionType.Identity,
                        bias=b2m)
                    w_sb[(i, m)] = wsb

            # scatter conv weights into zero-padded tables (SWDGE, flexible)
            # A tables: U[(s*3+kx)*255 + 127 - ky] = kernelA[s, ky, kx]
            for ky in range(3):
                nc.gpsimd.dma_start(
                    _sub_ap(tables, [[TBL, 3], [3 * TBL, S]],
                            BASE_A + 127 - ky),
                    vT[3 * ky:3 * ky + 3, :])
            # stage1/2: U[((s*3+m)*7+kx)*255 + 127 - ky] = w[i][s, m, ky, kx]
            for i, base in ((0, BASE_S1), (1, BASE_S2)):
                for m in range(ML):
                    for ky in range(KK):
                        nc.gpsimd.dma_start(
                            _sub_ap(tables, [[TBL, KK], [21 * TBL, S]],
                                    base + m * KK * TBL + 127 - ky),
                            w_sb[(i, m)][KK * ky:KK * ky + KK, :])

            # ---------------- main per-sample loop ----------------
            for s in range(S):
                # ---- flipped bands Bf[i, slot, m] = U_slot[i + m]
                bf = bfpool.tile([128, BF], F32, name=f"bf{s}", tag="bf")
                for (nslots, slot0, base) in (
                        (3, 0, BASE_A + s * 3 * TBL),
                        (21, 3, BASE_S1 + s * 21 * TBL),
                        (21, 24, BASE_S2 + s * 21 * TBL)):
                    nc.sync.dma_start(
                        _sub_ap(bf, [[BF, 128], [BANDW, nslots], [1, BANDW]],
                                slot0 * BANDW),
                        _sub_ap(tables, [[1, 128], [TBL, nslots], [1, BANDW]],
                                base))
                # ---- reverse partitions on PE: B[p] = Bf[127-p]
                bb = bpool.tile([128, BF], F32R if USE_F32R else F32, name=f"bb{s}", tag="bands")
                for c in range(0, BF, 512):
                    w = min(512, BF - c)
                    pr = psx.tile([128, 512], F32, name=f"pr{s}_{c}", tag="aux")
                    nc.tensor.matmul(pr[:, :w], rev, bf[:, c:c + w],
                                     start=True, stop=True)
                    nc.scalar.copy(bb[:, c:c + w], pr[:, :w])

                def band(slot, M):
                    return bb[:, slot * BANDW:slot * BANDW + M]

                # ---- Ax and r = f - Ax (126-row tiles) ----
                for (o0, M, row_start) in AX_TILES:
                    xt = xa_pool.tile([128, N + 2], F32R if USE_F32R else F32,
                                      name=f"xt{s}_{o0}", tag="xa")
                    if row_start + 128 > N:          # bottom tile: ones pad
                        nc.gpsimd.memset(xt.bitcast(F32), 1.0)
                        nd = N - row_start
                        nc.gpsimd.dma_start(xt[0:nd, 1:N + 1],
                                            x[s, row_start:N, :])
                        nc.gpsimd.memset(xt[0:nd, 0:1].bitcast(F32), 0.0)
                    else:
                        lo = max(0, row_start)
                        p0 = lo - row_start
                        if p0 > 0:
                            nc.gpsimd.memset(xt[0:p0, :].bitcast(F32), 0.0)
                        nc.gpsimd.dma_start(xt[p0:128, 1:N + 1],
                                            x[s, lo:row_start + 128, :])
                        nc.gpsimd.memset(xt[:, 0:1].bitcast(F32), 0.0)
                        nc.gpsimd.memset(xt[:, N + 1:N + 2].bitcast(F32), 1.0)
                    ps = psA.tile([M, N], F32, name=f"psA{s}_{o0}", tag="ax")
                    for kx in range(3):
                        nc.tensor.matmul(ps, _cast(band(_slot_a(kx), M)),
                                         _cast(xt[:, kx:kx + N]),
                                         start=(kx == 0), stop=(kx == 2))
                    ft = fr_pool.tile([126, N], F32, name=f"ft{s}_{o0}",
                                      tag="f")
                    nc.sync.dma_start(ft[:M, :], f[s, o0:o0 + M, :])
                    rt = fr_pool.tile([126, N], F32, name=f"rt{s}_{o0}",
                                      tag="r")
                    nc.vector.tensor_sub(rt[:M, :], ft[:M, :], ps)
                    nc.scalar.dma_start(r_dram[s, o0:o0 + M, :], rt[:M, :])

                # ---- stage 1: tmp_m = corr7(r, w1_m) ----
                for (o0, M, row_start) in S7_TILES:
                    rt7 = rhs_pool.tile([128, N + 6], F32R if USE_F32R else F32,
                                        name=f"rt7_{s}_{o0}", tag="rt7")
                    nc.gpsimd.memset(rt7.bitcast(F32), 0.0)
                    lo = max(0, row_start)
                    hi = min(N, row_start + 128)
                    nc.gpsimd.dma_start(
                        rt7[lo - row_start:hi - row_start, 3:N + 3],
                        r_dram[s, lo:hi, :])
                    tm3 = st_pool.tile([122, 3 * N], F32,
                                       name=f"tm3_{s}_{o0}", tag="tmp")
                    for m in range(ML):
                        ps = ps1.tile([M, N], F32, name=f"ps1_{s}_{o0}_{m}",
                                      tag="s1")
                        for kx in range(KK):
                            nc.tensor.matmul(ps, _cast(band(_slot_s1(m, kx), M)),
                                             _cast(rt7[:, kx:kx + N]),
                                             start=(kx == 0), stop=(kx == 6))
                        nc.vector.tensor_copy(tm3[:M, m * N:(m + 1) * N], ps)
                    # one store for all 3 maps: tmp_dram[s, :, o0:o0+M, :]
                    nc.scalar.dma_start(
                        _sub_ap(tmp_dram, [[N, M], [N * N, ML], [1, N]],
                                ((s * ML) * N + o0) * N),
                        _sub_ap(tm3, [[3 * N, M], [N, ML], [1, N]], 0))

                # ---- stage 2: G2 = sum_m corr7(tmp_m, w2_m); out = x + G2
                for (o0, M, row_start) in S7_TILES:
                    lo = max(0, row_start)
                    hi = min(N, row_start + 128)
                    tt = rhs_pool.tile([128, 3 * (N + 6)], F32R if USE_F32R else F32,
                                       name=f"tt{s}_{o0}", tag="tt")
                    nc.gpsimd.memset(tt.bitcast(F32), 0.0)
                    # one load for all 3 maps, each into its 518-block at col 3
                    nc.gpsimd.dma_start(
                        _sub_ap(tt, [[3 * (N + 6), hi - lo],
                                     [N + 6, ML], [1, N]],
                                (lo - row_start) * 3 * (N + 6) + 3),
                        _sub_ap(tmp_dram, [[N, hi - lo], [N * N, ML], [1, N]],
                                ((s * ML) * N + lo) * N))
                    pg = ps2.tile([M, N], F32, name=f"ps2_{s}_{o0}", tag="s2")
                    idx = 0
                    for m in range(ML):
                        for kx in range(KK):
                            nc.tensor.matmul(
                                pg, _cast(band(_slot_s2(m, kx), M)),
                                _cast(tt[:, m * (N + 6) + kx:m * (N + 6) + kx + N]),
                                start=(idx == 0), stop=(idx == 20))
                            idx += 1
                    x2 = fr_pool.tile([126, N], F32, name=f"x2_{s}_{o0}",
                                      tag="x2")
                    nc.sync.dma_start(x2[:M, :], x[s, o0:o0 + M, :])
                    ob = st_pool.tile([122, N], F32, name=f"ob{s}_{o0}",
                                      tag="ob")
                    nc.vector.tensor_add(ob[:M, :], x2[:M, :], pg)
                    nc.scalar.dma_start(out[s, o0:o0 + M, :], ob[:M, :])
    return nc


_CACHED = None


def _get_nc():
    global _CACHED
    if _CACHED is None:
        nc = bacc.Bacc("TRN2", debug=False, enable_asserts=False,
                       num_devices=NCORES)
        build_kernel(nc)
        nc.compile()
        _CACHED = nc
    return _CACHED


def make_in_maps(x, f, kernelA, fc1_w1, fc1_b1, fc1_w2, fc1_b2,
                 fc2_w1, fc2_b1, fc2_w2, fc2_b2):
    shared = {
        "fc1_w1": np.ascontiguousarray(fc1_w1, np.float32),
        "fc1_b1": np.ascontiguousarray(fc1_b1, np.float32),
        "fc1_w2": np.ascontiguousarray(fc1_w2, np.float32),
        "fc1_b2": np.ascontiguousarray(fc1_b2, np.float32),
        "fc2_w1": np.ascontiguousarray(fc2_w1, np.float32),
        "fc2_b1": np.ascontiguousarray(fc2_b1, np.float32),
        "fc2_w2": np.ascontiguousarray(fc2_w2, np.float32),
        "fc2_b2": np.ascontiguousarray(fc2_b2, np.float32),
    }
    in_maps = []
    for c in range(NCORES):
        sl = slice(S * c, S * (c + 1))
        in_maps.append({
            "x": np.ascontiguousarray(x[sl, 0], np.float32),
            "f": np.ascontiguousarray(f[sl, 0], np.float32),
            "kernelA": np.ascontiguousarray(
                kernelA[sl, 0].reshape(S, 9), np.float32),
            **shared,
        })
    return in_maps


def kernel(x, f, kernelA, fc1_w1, fc1_b1, fc1_w2, fc1_b2,
           fc2_w1, fc2_b1, fc2_w2, fc2_b2):
    x = np.asarray(x)
    nc = _get_nc()
    in_maps = make_in_maps(x, f, kernelA, fc1_w1, fc1_b1, fc1_w2, fc1_b2,
                           fc2_w1, fc2_b1, fc2_w2, fc2_b2)
    res = bass_utils.run_bass_kernel_spmd(
        nc, in_maps, core_ids=list(range(NCORES)))
    outs = [res.results[c]["out"] for c in range(NCORES)]
    full = np.concatenate(outs, axis=0).reshape(64, 1, N, N).astype(np.float32)
    return full
